# revision 27
# baseline (speedup 1.0000x reference)
"""HSIC pairwise loss kernel for trn2 (8 NeuronCores), fp8 DoubleRow.

Math: reference builds K_c = (w^2 w^2T) * (E_c E_c^T), M_c = R K_c, and sums
tr(M_i M_j) over i<j. With F_c = w^2 * E_c (row scaling), R the centering
matrix (idempotent):
    tr(R K_i R K_j) = ||G_i^T G_j||_F^2,  G_c = F_c - colmean(F_c)
and with A_ij = F_i^T F_j, s_c = F_c^T 1:
    G_i^T G_j = A_ij - (1/n) s_i s_j^T
so loss = sum_{i<j} ||A_ij - s_i s_j^T / n||_F^2 / (n-1)^2.

Device work: the 45 A_ij blocks [256,256] (contraction over n=4096) at
half-chunk granularity: 180 unordered cross-parent pairs of the 20
128-col units.  Decomposition: at the PARENT (chunk) level, cover K10's
45 edges with 8 bipartite K2,3 graphs (one per core, exactly 3 edges
double-covered).  Core c loads its 2 left parents (4 units, slots 0-3)
and 3 right parents (6 units, slots 4-9) = 10 units, 5.2 MB fp8, and
computes all 24 left x right unit blocks as six 512-wide DoubleRow
windows into 6 PSUM banks (24 blocks/core vs the 26 of the previous
window-search assignment; 22.5 is the absolute lower bound, and 23 is
provably unreachable for any SPMD one-hole bipartite template).

Input DMA rides BOTH HWDGE rings (sync/SP h0 + scalar/ACT h1,
concurrent FIFO rings).  A DMA's completion semaphore fires ~2.5-4us
AFTER its transfer ends (write-receipt pipeline, deeper under load),
so first-consumable data is ~10.2-10.5us regardless of transfer
speed: the PE warm-up dummy ramp (vector-memset zero tile, 7 wide + 6
narrow DR matmuls, ~3.6us) bridges from body entry to exactly then,
and HAM (needing ~3.42us of SUSTAINED activity) unthrottles the PE to
2.4 GHz just before the real matmuls start.  Super-tiles 0 and 1 ride
one batched transfer per ring so both are consumable when the ramp
ends; k>=2 go as singles for fine wait granularity (more/smaller
transfers starve the 4 completion lanes per ring - measured).  The
PSUM drain runs on vector only (no ACTIVATE -> no 1.3us
ACT_TABLE_LOAD blocking the ACT ring at start); the last 4 supertiles
are peeled window-major (stop cadence 864ns > 678ns drain) so drains
and 3 of the 4 output DMAs cascade DURING the final matmuls.  Host
pre-scales w^2*X by a power of two into fp8e4 (loss rel-err ~2e-3,
tolerance 2e-2), takes column sums for the rank-1 centering
correction, and assembles/reduces in float64.

Measured: ~37.0-37.5us HW exec (prev session's kernel: 40.5us).
Fixed costs in the measured window: ~4.8us from window start to
first-consumable data, ~2.4us final-output-DMA completion wait, and
~7us NEFF per-engine semaphore-file zeroing epilogue + barriers
(lowering-emitted, present in any kernel).  The 20.9us matmul phase
is fp8-DoubleRow peak (157 TF/s) x 24/22.5 cover overhead.
"""

import numpy as np
import ml_dtypes
from contextlib import ExitStack

import concourse.bass as bass
import concourse.tile as tile
from concourse import bacc, mybir
from concourse import bass_utils

N = 4096
KT = 16                      # k super-tiles of 256 rows (DoubleRow)
UNITS = 10                   # units (128-col half-chunks) per core
ROW = UNITS * 128            # 1280 data cols
WARM_MMS = 7                 # wide dummy matmuls to pre-warm the PE (HAM)
WARM_MMS_NARROW = 6          # fine-grained tail of the warm-up ramp
PEEL = 4                     # trailing super-tiles run window-major so the
                             # PSUM drains cascade under the final matmuls

# Parent-level K2,3 cover of K10: core c loads left parents L (slots
# 0-3) and right parents R (slots 4-9); covers all L x R unit pairs.
COVER = [
    ((5, 7), (6, 4, 3)),
    ((7, 6), (2, 0, 8)),
    ((4, 5), (7, 1, 2)),
    ((6, 8), (4, 3, 5)),
    ((1, 9), (0, 6, 7)),
    ((8, 3), (4, 0, 1)),
    ((9, 0), (4, 2, 5)),
    ((9, 2), (3, 8, 1)),
]
ASSIGN = [
    [2 * p + h for p in L + R for h in (0, 1)] for (L, R) in COVER
]

# (stat_slot, moving_start_col, n_cols): slots 0-3 against slots 4-7,
# slots 8 and 9 against slots 0-3  ->  full K4,6 = 24 blocks.  The last
# 512 window is split 384+128 so the FINAL window's PSUM drain + output
# DMA (which sit after the last matmul on the critical path, ahead of a
# ~2.4us DMA-completion wait the epilogue barriers gate on) are tiny.
WINDOWS = [
    (0, 512, 512),
    (1, 512, 512),
    (2, 512, 512),
    (3, 512, 512),
    (8, 0, 512),
    (9, 0, 384),
    (9, 384, 128),
]
OUT_COLS = sum(w[2] for w in WINDOWS)   # 3072

_CACHE = {}


def _build(c_out):
    f32 = mybir.dt.float32
    f8 = mybir.dt.float8e4
    DR = mybir.MatmulPerfMode.DoubleRow
    nc = bacc.Bacc("TRN2", target_bir_lowering=False, debug=False,
                   num_devices=8)
    # input layout [half, partition, supertile, col]: lets one DMA carry
    # one half of several supertiles as a single strided 3D transfer.
    x = nc.dram_tensor("x", [2, 128, KT, ROW], f8, kind="ExternalInput").ap()
    out = nc.dram_tensor("out", [128, OUT_COLS], f8,
                         kind="ExternalOutput").ap()

    with tile.TileContext(nc) as tc:
        with ExitStack() as ctx:
            zpool = ctx.enter_context(tc.tile_pool(name="z", bufs=1))
            xpool = ctx.enter_context(tc.tile_pool(name="xs", bufs=1))
            psum = ctx.enter_context(tc.tile_pool(name="ps", bufs=1,
                                                  space="PSUM"))
            opool = ctx.enter_context(tc.tile_pool(name="o", bufs=1))

            ps = []
            for i, (_, _, nw) in enumerate(WINDOWS):
                pst = psum.tile([128, nw], f32, tag=f"ps{i}", name=f"ps{i}")
                ps.append(pst)

            # PE warm-up: dummy DoubleRow matmuls start the HAM activity
            # window (~3.4us to unthrottle 1.2 -> 2.4 GHz).  The input
            # DMA completion semaphores fire only ~2.2-2.6us after the
            # transfer ends (~10.2-10.5us), so the ramp bridges from
            # body entry until then.  Products land in ps[0] and are
            # discarded (the real start=True matmul resets it).
            zt = zpool.tile([128, 2, 128], f8, tag="zt")
            nc.vector.memset(zt[:], 0.0)
            zr = zt[:, :, :].unsqueeze(2).broadcast_to([128, 2, 4, 128])
            for _ in range(WARM_MMS):
                nc.tensor.matmul(ps[0][:, 0:512], zt[:, :, :], zr,
                                 start=True, stop=True, perf_mode=DR)
            for _ in range(WARM_MMS_NARROW):
                nc.tensor.matmul(ps[0][:, 0:128], zt[:, :, :], zt[:, :, :],
                                 start=True, stop=True, perf_mode=DR)

            # Input: supertile halves on the two concurrent HWDGE rings
            # (sync h0 + scalar h1).  Super-tiles 0 and 1 ride ONE
            # transfer per ring (their completion semaphores fire
            # ~2.5-4us after the transfer ends, and a ring's 2nd
            # transfer completes ~1.9us after its 1st -- batching makes
            # BOTH supertiles consumable right as the warm-up ramp ends
            # instead of stalling the PE after supertile 0).  k>=2 go as
            # singles so the matmul wait granularity stays fine; more or
            # smaller transfers only deepen the completion queue and
            # starve the 4 in-flight lanes per ring (measured).
            f01 = xpool.tile([128, 2, 2, ROW], f8, name="f01")
            nc.sync.dma_start(f01[:, 0, :, :], x[0, :, 0:2, :])
            nc.scalar.dma_start(f01[:, 1, :, :], x[1, :, 0:2, :])
            fts = [f01[:, :, 0, :], f01[:, :, 1, :]]
            for k in range(2, KT):
                ft = xpool.tile([128, 2, ROW], f8, name=f"ft{k}")
                nc.sync.dma_start(ft[:, 0, :], x[0, :, k, :])
                nc.scalar.dma_start(ft[:, 1, :], x[1, :, k, :])
                fts.append(ft[:, :, :])
            for k in range(KT - PEEL):
                ft = fts[k]
                for wi, (s, mc, nw) in enumerate(WINDOWS):
                    nc.tensor.matmul(
                        ps[wi][:, 0:nw],
                        ft[:, :, s * 128:(s + 1) * 128],
                        ft[:, :, mc:mc + nw],
                        start=(k == 0),
                        stop=False,
                        perf_mode=DR,
                    )
            for wi, (s, mc, nw) in enumerate(WINDOWS):
                for k in range(KT - PEEL, KT):
                    ft = fts[k]
                    nc.tensor.matmul(
                        ps[wi][:, 0:nw],
                        ft[:, :, s * 128:(s + 1) * 128],
                        ft[:, :, mc:mc + nw],
                        start=False,
                        stop=(k == KT - 1),
                        perf_mode=DR,
                    )

            # PSUM -> SBUF on vector only, scaled into fp8 range (c_out
            # is a power of two picked on the host so |A|*c_out <= 224
            # by Cauchy-Schwarz); three output DMAs so transfers overlap
            # the remaining copies.
            ot = opool.tile([128, OUT_COLS], f8)
            col = 0
            cuts = []
            for wi, (s, mc, nw) in enumerate(WINDOWS):
                nc.vector.tensor_scalar_mul(ot[:, col:col + nw],
                                            ps[wi][:, 0:nw], c_out)
                col += nw
                if wi in (1, 3, 5, 6):
                    cuts.append(col)
            lo = 0
            for hi in cuts:
                nc.sync.dma_start(out[:, lo:hi], ot[:, lo:hi])
                lo = hi
    nc.compile()
    return nc


def _get_nc(c_out):
    if _CACHE.get("c_out") != c_out:
        _CACHE["nc"] = _build(c_out)
        _CACHE["c_out"] = c_out
    return _CACHE["nc"]


def _quantize(X, w):
    """Host prep: F = w^2 * X, scaled by a power of two into fp8e4 range."""
    F = (w.astype(np.float64) ** 2) * X.astype(np.float64)
    amax = float(np.abs(F).max())
    if amax == 0.0 or not np.isfinite(amax):
        scale = 1.0
    else:
        scale = 2.0 ** np.floor(np.log2(192.0 / amax))
    Fq = np.clip(F * scale, -240.0, 240.0).astype(ml_dtypes.float8_e4m3)
    return Fq, scale


def _in_maps(Fq):
    maps = []
    for units in ASSIGN:
        xc = np.concatenate([Fq[:, u * 128:(u + 1) * 128] for u in units],
                            axis=1)
        # device layout [half, partition, supertile, col]
        xd = np.ascontiguousarray(
            xc.reshape(KT, 2, 128, ROW).transpose(1, 2, 0, 3))
        maps.append({"x": xd})
    return maps


def _assemble(outs, svec, scale, c_out):
    inv = 1.0 / (scale * scale * c_out)
    quad = {}
    for c, units in enumerate(ASSIGN):
        o = outs[c].astype(np.float64) * inv
        col = 0
        for (s, mc, nw) in WINDOWS:
            su = units[s]
            block = o[:, col:col + nw]
            col += nw
            m0 = mc // 128
            for t in range(nw // 128):
                quad[(su, units[m0 + t])] = block[:, t * 128:(t + 1) * 128]
    loss = 0.0
    for i in range(10):
        s_i = np.concatenate([svec[2 * i], svec[2 * i + 1]])
        for j in range(i + 1, 10):
            s_j = np.concatenate([svec[2 * j], svec[2 * j + 1]])
            A = np.empty((256, 256))
            for a in range(2):
                for b in range(2):
                    u, v = 2 * i + a, 2 * j + b
                    q = quad[(u, v)] if (u, v) in quad else quad[(v, u)].T
                    A[a * 128:(a + 1) * 128, b * 128:(b + 1) * 128] = q
            C = A - np.outer(s_i, s_j) / float(N)
            loss += float((C * C).sum())
    loss /= float((N - 1) * (N - 1))
    return np.asarray([loss], np.float32)


def kernel(final_readout, weight, _trace=False):
    X = np.ascontiguousarray(np.asarray(final_readout, np.float32))
    w = np.asarray(weight, np.float32)
    Fq, scale = _quantize(X, w)
    # column sums of the quantized data (exact, fp64) for the centering
    # correction; must match the data the device saw.
    Fq64 = Fq.astype(np.float64)
    scol = Fq64.sum(axis=0) / scale
    svec = {u: scol[u * 128:(u + 1) * 128] for u in range(20)}
    # output rescale: |A_q| <= max col norm squared (Cauchy-Schwarz), so
    # A_q * c_out fits fp8e4 range with no clipping
    cn2 = float((Fq64 * Fq64).sum(axis=0).max())
    c_out = 2.0 ** np.floor(np.log2(224.0 / cn2)) if cn2 > 0 else 1.0
    nc = _get_nc(c_out)
    res = bass_utils.run_bass_kernel_spmd(
        nc, _in_maps(Fq), core_ids=list(range(8)), trace=_trace)
    _CACHE["last_results"] = res
    return _assemble([r["out"] for r in res.results], svec, scale, c_out)


# revision 28
# speedup vs baseline: 1.0072x; 1.0072x over previous
"""HSIC pairwise loss kernel for trn2 (8 NeuronCores), fp8 DoubleRow.

Math: reference builds K_c = (w^2 w^2T) * (E_c E_c^T), M_c = R K_c, and sums
tr(M_i M_j) over i<j. With F_c = w^2 * E_c (row scaling), R the centering
matrix (idempotent):
    tr(R K_i R K_j) = ||G_i^T G_j||_F^2,  G_c = F_c - colmean(F_c)
and with A_ij = F_i^T F_j, s_c = F_c^T 1:
    G_i^T G_j = A_ij - (1/n) s_i s_j^T
so loss = sum_{i<j} ||A_ij - s_i s_j^T / n||_F^2 / (n-1)^2.

Device work: the 45 A_ij blocks [256,256] (contraction over n=4096) at
half-chunk granularity: 180 unordered cross-parent pairs of the 20
128-col units.  Decomposition: at the PARENT (chunk) level, cover K10's
45 edges with 8 bipartite K2,3 graphs (one per core, exactly 3 edges
double-covered).  Core c loads its 2 left parents (4 units, slots 0-3)
and 3 right parents (6 units, slots 4-9) = 10 units, 5.2 MB fp8, and
computes all 24 left x right unit blocks as six 512-wide DoubleRow
windows into 6 PSUM banks (24 blocks/core vs the 26 of the previous
window-search assignment; 22.5 is the absolute lower bound, and 23 is
provably unreachable for any SPMD one-hole bipartite template).

Input DMA rides BOTH HWDGE rings (sync/SP h0 + scalar/ACT h1,
concurrent FIFO rings).  A DMA's completion semaphore fires ~2.5-4us
AFTER its transfer ends (write-receipt pipeline, deeper under load),
so first-consumable data is ~10.2-10.5us regardless of transfer
speed: the PE warm-up dummy ramp (vector-memset zero tile, 7 wide + 6
narrow DR matmuls, ~3.6us) bridges from body entry to exactly then,
and HAM (needing ~3.42us of SUSTAINED activity) unthrottles the PE to
2.4 GHz just before the real matmuls start.  Super-tiles 0 and 1 ride
one batched transfer per ring so both are consumable when the ramp
ends; k>=2 go as singles for fine wait granularity (more/smaller
transfers starve the 4 completion lanes per ring - measured).  The
PSUM drain runs on vector only (no ACTIVATE -> no 1.3us
ACT_TABLE_LOAD blocking the ACT ring at start); the last 4 supertiles
are peeled window-major (stop cadence 864ns > 678ns drain) so drains
and 3 of the 4 output DMAs cascade DURING the final matmuls.  Host
pre-scales w^2*X by a power of two into fp8e4 (loss rel-err ~2e-3,
tolerance 2e-2), takes column sums for the rank-1 centering
correction, and assembles/reduces in float64.

Measured: ~37.0-37.5us HW exec (prev session's kernel: 40.5us).
Fixed costs in the measured window: ~4.8us from window start to
first-consumable data, ~2.4us final-output-DMA completion wait, and
~7us NEFF per-engine semaphore-file zeroing epilogue + barriers
(lowering-emitted, present in any kernel).  The 20.9us matmul phase
is fp8-DoubleRow peak (157 TF/s) x 24/22.5 cover overhead.
"""

import numpy as np
import ml_dtypes
from contextlib import ExitStack

import concourse.bass as bass
import concourse.tile as tile
from concourse import bacc, mybir
from concourse import bass_utils

N = 4096
KT = 16                      # k super-tiles of 256 rows (DoubleRow)
UNITS = 10                   # units (128-col half-chunks) per core
ROW = UNITS * 128            # 1280 data cols
WARM_MMS = 7                 # wide dummy matmuls to pre-warm the PE (HAM)
WARM_MMS_NARROW = 6          # fine-grained tail of the warm-up ramp
PEEL = 4                     # trailing super-tiles run window-major so the
                             # PSUM drains cascade under the final matmuls

# Parent-level K2,3 cover of K10: core c loads left parents L (slots
# 0-3) and right parents R (slots 4-9); covers all L x R unit pairs.
COVER = [
    ((5, 7), (6, 4, 3)),
    ((7, 6), (2, 0, 8)),
    ((4, 5), (7, 1, 2)),
    ((6, 8), (4, 3, 5)),
    ((1, 9), (0, 6, 7)),
    ((8, 3), (4, 0, 1)),
    ((9, 0), (4, 2, 5)),
    ((9, 2), (3, 8, 1)),
]
ASSIGN = [
    [2 * p + h for p in L + R for h in (0, 1)] for (L, R) in COVER
]

# (stat_slot, moving_start_col, n_cols): slots 0-3 against slots 4-7,
# slots 8 and 9 against slots 0-3  ->  full K4,6 = 24 blocks.  The last
# 512 window is split 384+128 so the FINAL window's PSUM drain + output
# DMA (which sit after the last matmul on the critical path, ahead of a
# ~2.4us DMA-completion wait the epilogue barriers gate on) are tiny.
WINDOWS = [
    (0, 512, 512),
    (1, 512, 512),
    (2, 512, 512),
    (3, 512, 512),
    (8, 0, 512),
    (9, 0, 384),
    (9, 384, 128),
]
OUT_COLS = sum(w[2] for w in WINDOWS)   # 3072

_CACHE = {}


def _build(c_out):
    f32 = mybir.dt.float32
    f8 = mybir.dt.float8e4
    DR = mybir.MatmulPerfMode.DoubleRow
    nc = bacc.Bacc("TRN2", target_bir_lowering=False, debug=False,
                   num_devices=8)
    # input layout [half, partition, supertile, col]: lets one DMA carry
    # one half of several supertiles as a single strided 3D transfer.
    x = nc.dram_tensor("x", [2, 128, KT, ROW], f8, kind="ExternalInput").ap()
    out = nc.dram_tensor("out", [128, OUT_COLS], f8,
                         kind="ExternalOutput").ap()

    with tile.TileContext(nc) as tc:
        with ExitStack() as ctx:
            zpool = ctx.enter_context(tc.tile_pool(name="z", bufs=1))
            xpool = ctx.enter_context(tc.tile_pool(name="xs", bufs=1))
            psum = ctx.enter_context(tc.tile_pool(name="ps", bufs=1,
                                                  space="PSUM"))
            opool = ctx.enter_context(tc.tile_pool(name="o", bufs=1))

            ps = []
            for i, (_, _, nw) in enumerate(WINDOWS):
                pst = psum.tile([128, nw], f32, tag=f"ps{i}", name=f"ps{i}")
                ps.append(pst)

            # PE warm-up: dummy DoubleRow matmuls start the HAM activity
            # window (~3.4us to unthrottle 1.2 -> 2.4 GHz).  The input
            # DMA completion semaphores fire only ~2.2-2.6us after the
            # transfer ends (~10.2-10.5us), so the ramp bridges from
            # body entry until then.  Products land in ps[0] and are
            # discarded (the real start=True matmul resets it).
            zt = zpool.tile([128, 2, 128], f8, tag="zt")
            nc.vector.memset(zt[:], 0.0)
            zr = zt[:, :, :].unsqueeze(2).broadcast_to([128, 2, 4, 128])
            for _ in range(WARM_MMS):
                nc.tensor.matmul(ps[0][:, 0:512], zt[:, :, :], zr,
                                 start=True, stop=True, perf_mode=DR)
            for _ in range(WARM_MMS_NARROW):
                nc.tensor.matmul(ps[0][:, 0:128], zt[:, :, :], zt[:, :, :],
                                 start=True, stop=True, perf_mode=DR)

            # Input: supertile halves on the two concurrent HWDGE rings
            # (sync h0 + scalar h1).  Super-tiles 0 and 1 ride ONE
            # transfer per ring (their completion semaphores fire
            # ~2.5-4us after the transfer ends, and a ring's 2nd
            # transfer completes ~1.9us after its 1st -- batching makes
            # BOTH supertiles consumable right as the warm-up ramp ends
            # instead of stalling the PE after supertile 0).  k>=2 go as
            # singles so the matmul wait granularity stays fine; more or
            # smaller transfers only deepen the completion queue and
            # starve the 4 in-flight lanes per ring (measured).
            f01 = xpool.tile([128, 2, 2, ROW], f8, name="f01")
            nc.sync.dma_start(f01[:, 0, :, :], x[0, :, 0:2, :])
            nc.scalar.dma_start(f01[:, 1, :, :], x[1, :, 0:2, :])
            fts = [f01[:, :, 0, :], f01[:, :, 1, :]]
            for k in range(2, KT):
                ft = xpool.tile([128, 2, ROW], f8, name=f"ft{k}")
                nc.sync.dma_start(ft[:, 0, :], x[0, :, k, :])
                nc.scalar.dma_start(ft[:, 1, :], x[1, :, k, :])
                fts.append(ft[:, :, :])
            for k in range(KT - PEEL):
                ft = fts[k]
                for wi, (s, mc, nw) in enumerate(WINDOWS):
                    nc.tensor.matmul(
                        ps[wi][:, 0:nw],
                        ft[:, :, s * 128:(s + 1) * 128],
                        ft[:, :, mc:mc + nw],
                        start=(k == 0),
                        stop=False,
                        perf_mode=DR,
                    )
            for wi, (s, mc, nw) in enumerate(WINDOWS):
                for k in range(KT - PEEL, KT):
                    ft = fts[k]
                    nc.tensor.matmul(
                        ps[wi][:, 0:nw],
                        ft[:, :, s * 128:(s + 1) * 128],
                        ft[:, :, mc:mc + nw],
                        start=False,
                        stop=(k == KT - 1),
                        perf_mode=DR,
                    )

            # PSUM -> SBUF on vector only, scaled into fp8 range (c_out
            # is a power of two picked on the host so |A|*c_out <= 224
            # by Cauchy-Schwarz); three output DMAs so transfers overlap
            # the remaining copies.
            # Output cuts: the first three ride the sync ring while the
            # peel is still computing; the FINAL tiny cut (128 cols,
            # after the last matmul) goes on the otherwise-idle scalar
            # ring so its trigger is not queued behind a sync-ring
            # trigger still in flight (~600ns floor each).
            ot = opool.tile([128, OUT_COLS], f8)
            col = 0
            cuts = []
            for wi, (s, mc, nw) in enumerate(WINDOWS):
                nc.vector.tensor_scalar_mul(ot[:, col:col + nw],
                                            ps[wi][:, 0:nw], c_out)
                col += nw
                if wi in (1, 3, 5, 6):
                    cuts.append(col)
            lo = 0
            for ci, hi in enumerate(cuts):
                eng = nc.scalar if ci == len(cuts) - 1 else nc.sync
                eng.dma_start(out[:, lo:hi], ot[:, lo:hi])
                lo = hi
    nc.compile()
    return nc


def _get_nc(c_out):
    if _CACHE.get("c_out") != c_out:
        _CACHE["nc"] = _build(c_out)
        _CACHE["c_out"] = c_out
    return _CACHE["nc"]


def _quantize(X, w):
    """Host prep: F = w^2 * X, scaled by a power of two into fp8e4 range."""
    F = (w.astype(np.float64) ** 2) * X.astype(np.float64)
    amax = float(np.abs(F).max())
    if amax == 0.0 or not np.isfinite(amax):
        scale = 1.0
    else:
        scale = 2.0 ** np.floor(np.log2(192.0 / amax))
    Fq = np.clip(F * scale, -240.0, 240.0).astype(ml_dtypes.float8_e4m3)
    return Fq, scale


def _in_maps(Fq):
    maps = []
    for units in ASSIGN:
        xc = np.concatenate([Fq[:, u * 128:(u + 1) * 128] for u in units],
                            axis=1)
        # device layout [half, partition, supertile, col]
        xd = np.ascontiguousarray(
            xc.reshape(KT, 2, 128, ROW).transpose(1, 2, 0, 3))
        maps.append({"x": xd})
    return maps


def _assemble(outs, svec, scale, c_out):
    inv = 1.0 / (scale * scale * c_out)
    quad = {}
    for c, units in enumerate(ASSIGN):
        o = outs[c].astype(np.float64) * inv
        col = 0
        for (s, mc, nw) in WINDOWS:
            su = units[s]
            block = o[:, col:col + nw]
            col += nw
            m0 = mc // 128
            for t in range(nw // 128):
                quad[(su, units[m0 + t])] = block[:, t * 128:(t + 1) * 128]
    loss = 0.0
    for i in range(10):
        s_i = np.concatenate([svec[2 * i], svec[2 * i + 1]])
        for j in range(i + 1, 10):
            s_j = np.concatenate([svec[2 * j], svec[2 * j + 1]])
            A = np.empty((256, 256))
            for a in range(2):
                for b in range(2):
                    u, v = 2 * i + a, 2 * j + b
                    q = quad[(u, v)] if (u, v) in quad else quad[(v, u)].T
                    A[a * 128:(a + 1) * 128, b * 128:(b + 1) * 128] = q
            C = A - np.outer(s_i, s_j) / float(N)
            loss += float((C * C).sum())
    loss /= float((N - 1) * (N - 1))
    return np.asarray([loss], np.float32)


def kernel(final_readout, weight, _trace=False):
    X = np.ascontiguousarray(np.asarray(final_readout, np.float32))
    w = np.asarray(weight, np.float32)
    Fq, scale = _quantize(X, w)
    # column sums of the quantized data (exact, fp64) for the centering
    # correction; must match the data the device saw.
    Fq64 = Fq.astype(np.float64)
    scol = Fq64.sum(axis=0) / scale
    svec = {u: scol[u * 128:(u + 1) * 128] for u in range(20)}
    # output rescale: |A_q| <= max col norm squared (Cauchy-Schwarz), so
    # A_q * c_out fits fp8e4 range with no clipping
    cn2 = float((Fq64 * Fq64).sum(axis=0).max())
    c_out = 2.0 ** np.floor(np.log2(224.0 / cn2)) if cn2 > 0 else 1.0
    nc = _get_nc(c_out)
    res = bass_utils.run_bass_kernel_spmd(
        nc, _in_maps(Fq), core_ids=list(range(8)), trace=_trace)
    _CACHE["last_results"] = res
    return _assemble([r["out"] for r in res.results], svec, scale, c_out)


# revision 29
# speedup vs baseline: 1.0226x; 1.0153x over previous
"""HSIC pairwise loss kernel for trn2 (8 NeuronCores), fp8 DoubleRow.

Math: reference builds K_c = (w^2 w^2T) * (E_c E_c^T), M_c = R K_c, and sums
tr(M_i M_j) over i<j. With F_c = w^2 * E_c (row scaling), R the centering
matrix (idempotent):
    tr(R K_i R K_j) = ||G_i^T G_j||_F^2,  G_c = F_c - colmean(F_c)
and with A_ij = F_i^T F_j, s_c = F_c^T 1:
    G_i^T G_j = A_ij - (1/n) s_i s_j^T
so loss = sum_{i<j} ||A_ij - s_i s_j^T / n||_F^2 / (n-1)^2.

Device work: the 45 A_ij blocks [256,256] (contraction over n=4096) at
half-chunk granularity: 180 unordered cross-parent pairs of the 20
128-col units.  Decomposition: at the PARENT (chunk) level, cover K10's
45 edges with 8 bipartite K2,3 graphs (one per core, exactly 3 edges
double-covered).  Core c loads its 2 left parents (4 units, slots 0-3)
and 3 right parents (6 units, slots 4-9) = 10 units, 5.2 MB fp8, and
computes all 24 left x right unit blocks as six 512-wide DoubleRow
windows into 6 PSUM banks (24 blocks/core vs the 26 of the previous
window-search assignment; 22.5 is the absolute lower bound, and 23 is
provably unreachable for any SPMD one-hole bipartite template).

Input DMA rides BOTH HWDGE rings (sync/SP h0 + scalar/ACT h1,
concurrent FIFO rings).  A DMA's completion semaphore fires ~2.5-4us
AFTER its transfer ends (write-receipt pipeline, deeper under load),
so first-consumable data is ~10.2-10.5us regardless of transfer
speed: the PE warm-up dummy ramp (vector-memset zero tile, 7 wide + 6
narrow DR matmuls, ~3.6us) bridges from body entry to exactly then,
and HAM (needing ~3.42us of SUSTAINED activity) unthrottles the PE to
2.4 GHz just before the real matmuls start.  Super-tiles 0 and 1 ride
one batched transfer per ring so both are consumable when the ramp
ends; k>=2 go as singles for fine wait granularity (more/smaller
transfers starve the 4 completion lanes per ring - measured).  The
PSUM drain runs on vector only (no ACTIVATE -> no 1.3us
ACT_TABLE_LOAD blocking the ACT ring at start); the last 4 supertiles
are peeled window-major (stop cadence 864ns > 678ns drain) so drains
and 3 of the 4 output DMAs cascade DURING the final matmuls.  Host
pre-scales w^2*X by a power of two into fp8e4 (loss rel-err ~2e-3,
tolerance 2e-2), takes column sums for the rank-1 centering
correction, and assembles/reduces in float64.

Measured: ~37.0-37.5us HW exec (prev session's kernel: 40.5us).
Fixed costs in the measured window: ~4.8us from window start to
first-consumable data, ~2.4us final-output-DMA completion wait, and
~7us NEFF per-engine semaphore-file zeroing epilogue + barriers
(lowering-emitted, present in any kernel).  The 20.9us matmul phase
is fp8-DoubleRow peak (157 TF/s) x 24/22.5 cover overhead.
"""

import numpy as np
import ml_dtypes
from contextlib import ExitStack

import concourse.bass as bass
import concourse.tile as tile
from concourse import bacc, mybir
from concourse import bass_utils

N = 4096
KT = 16                      # k super-tiles of 256 rows (DoubleRow)
UNITS = 10                   # units (128-col half-chunks) per core
ROW = UNITS * 128            # 1280 data cols
WARM_MMS = 7                 # wide dummy matmuls to pre-warm the PE (HAM)
WARM_MMS_NARROW = 6          # fine-grained tail of the warm-up ramp
PEEL = 4                     # trailing super-tiles run window-major so the
                             # PSUM drains cascade under the final matmuls

# Parent-level K2,3 cover of K10: core c loads left parents L (slots
# 0-3) and right parents R (slots 4-9); covers all L x R unit pairs.
COVER = [
    ((5, 7), (6, 4, 3)),
    ((7, 6), (2, 0, 8)),
    ((4, 5), (7, 1, 2)),
    ((6, 8), (4, 3, 5)),
    ((1, 9), (0, 6, 7)),
    ((8, 3), (4, 0, 1)),
    ((9, 0), (4, 2, 5)),
    ((9, 2), (3, 8, 1)),
]
ASSIGN = [
    [2 * p + h for p in L + R for h in (0, 1)] for (L, R) in COVER
]

# (stat_slot, moving_start_col, n_cols): slots 0-3 against slots 4-7,
# slots 8 and 9 against slots 0-3  ->  full K4,6 = 24 blocks.  The last
# 512 window is split 384+128 so the FINAL window's PSUM drain + output
# DMA (which sit after the last matmul on the critical path, ahead of a
# ~2.4us DMA-completion wait the epilogue barriers gate on) are tiny.
WINDOWS = [
    (0, 512, 512),
    (1, 512, 512),
    (2, 512, 512),
    (3, 512, 512),
    (8, 0, 512),
    (9, 0, 384),
    (3, 1152, 128),   # pair (3,9) with stationary 3: a DIFFERENT weight
                      # than w5's, so its LDWEIGHTS pipelines instead of
                      # serializing behind w5's matmul (same-address
                      # LDWEIGHTS cannot be pulled ahead)
]
OUT_COLS = sum(w[2] for w in WINDOWS)   # 3072

_CACHE = {}


def _build(c_out):
    f32 = mybir.dt.float32
    f8 = mybir.dt.float8e4
    DR = mybir.MatmulPerfMode.DoubleRow
    nc = bacc.Bacc("TRN2", target_bir_lowering=False, debug=False,
                   num_devices=8)
    # input layout [half, partition, supertile, col]: lets one DMA carry
    # one half of several supertiles as a single strided 3D transfer.
    x = nc.dram_tensor("x", [2, 128, KT, ROW], f8, kind="ExternalInput").ap()
    out = nc.dram_tensor("out", [128, OUT_COLS], f8,
                         kind="ExternalOutput").ap()

    with tile.TileContext(nc) as tc:
        with ExitStack() as ctx:
            zpool = ctx.enter_context(tc.tile_pool(name="z", bufs=1))
            xpool = ctx.enter_context(tc.tile_pool(name="xs", bufs=1))
            psum = ctx.enter_context(tc.tile_pool(name="ps", bufs=1,
                                                  space="PSUM"))
            opool = ctx.enter_context(tc.tile_pool(name="o", bufs=1))

            ps = []
            for i, (_, _, nw) in enumerate(WINDOWS):
                pst = psum.tile([128, nw], f32, tag=f"ps{i}", name=f"ps{i}")
                ps.append(pst)

            # PE warm-up: dummy DoubleRow matmuls start the HAM activity
            # window (~3.4us to unthrottle 1.2 -> 2.4 GHz).  The input
            # DMA completion semaphores fire only ~2.2-2.6us after the
            # transfer ends (~10.2-10.5us), so the ramp bridges from
            # body entry until then.  Products land in ps[0] and are
            # discarded (the real start=True matmul resets it).
            zt = zpool.tile([128, 2, 128], f8, tag="zt")
            nc.vector.memset(zt[:], 0.0)
            zr = zt[:, :, :].unsqueeze(2).broadcast_to([128, 2, 4, 128])
            for _ in range(WARM_MMS):
                nc.tensor.matmul(ps[0][:, 0:512], zt[:, :, :], zr,
                                 start=True, stop=True, perf_mode=DR)
            for _ in range(WARM_MMS_NARROW):
                nc.tensor.matmul(ps[0][:, 0:128], zt[:, :, :], zt[:, :, :],
                                 start=True, stop=True, perf_mode=DR)

            # Input: supertile halves on the two concurrent HWDGE rings
            # (sync h0 + scalar h1).  Super-tiles 0 and 1 ride ONE
            # transfer per ring (their completion semaphores fire
            # ~2.5-4us after the transfer ends, and a ring's 2nd
            # transfer completes ~1.9us after its 1st -- batching makes
            # BOTH supertiles consumable right as the warm-up ramp ends
            # instead of stalling the PE after supertile 0).  k>=2 go as
            # singles so the matmul wait granularity stays fine; more or
            # smaller transfers only deepen the completion queue and
            # starve the 4 in-flight lanes per ring (measured).
            f01 = xpool.tile([128, 2, 2, ROW], f8, name="f01")
            nc.sync.dma_start(f01[:, 0, :, :], x[0, :, 0:2, :])
            nc.scalar.dma_start(f01[:, 1, :, :], x[1, :, 0:2, :])
            fts = [f01[:, :, 0, :], f01[:, :, 1, :]]
            for k in range(2, KT):
                ft = xpool.tile([128, 2, ROW], f8, name=f"ft{k}")
                nc.sync.dma_start(ft[:, 0, :], x[0, :, k, :])
                nc.scalar.dma_start(ft[:, 1, :], x[1, :, k, :])
                fts.append(ft[:, :, :])
            for k in range(KT - PEEL):
                ft = fts[k]
                for wi, (s, mc, nw) in enumerate(WINDOWS):
                    nc.tensor.matmul(
                        ps[wi][:, 0:nw],
                        ft[:, :, s * 128:(s + 1) * 128],
                        ft[:, :, mc:mc + nw],
                        start=(k == 0),
                        stop=False,
                        perf_mode=DR,
                    )
            for wi, (s, mc, nw) in enumerate(WINDOWS):
                for k in range(KT - PEEL, KT):
                    ft = fts[k]
                    nc.tensor.matmul(
                        ps[wi][:, 0:nw],
                        ft[:, :, s * 128:(s + 1) * 128],
                        ft[:, :, mc:mc + nw],
                        start=False,
                        stop=(k == KT - 1),
                        perf_mode=DR,
                    )

            # PSUM -> SBUF on vector only, scaled into fp8 range (c_out
            # is a power of two picked on the host so |A|*c_out <= 224
            # by Cauchy-Schwarz); three output DMAs so transfers overlap
            # the remaining copies.
            # Output cuts: the first three ride the sync ring while the
            # peel is still computing; the FINAL tiny cut (128 cols,
            # after the last matmul) goes on the otherwise-idle scalar
            # ring so its trigger is not queued behind a sync-ring
            # trigger still in flight (~600ns floor each).
            ot = opool.tile([128, OUT_COLS], f8)
            col = 0
            cuts = []
            for wi, (s, mc, nw) in enumerate(WINDOWS):
                nc.vector.tensor_scalar_mul(ot[:, col:col + nw],
                                            ps[wi][:, 0:nw], c_out)
                col += nw
                if wi in (1, 3, 5, 6):
                    cuts.append(col)
            lo = 0
            for ci, hi in enumerate(cuts):
                eng = nc.scalar if ci == len(cuts) - 1 else nc.sync
                eng.dma_start(out[:, lo:hi], ot[:, lo:hi])
                lo = hi
    nc.compile()
    return nc


def _get_nc(c_out):
    if _CACHE.get("c_out") != c_out:
        _CACHE["nc"] = _build(c_out)
        _CACHE["c_out"] = c_out
    return _CACHE["nc"]


def _quantize(X, w):
    """Host prep: F = w^2 * X, scaled by a power of two into fp8e4 range."""
    F = (w.astype(np.float64) ** 2) * X.astype(np.float64)
    amax = float(np.abs(F).max())
    if amax == 0.0 or not np.isfinite(amax):
        scale = 1.0
    else:
        scale = 2.0 ** np.floor(np.log2(192.0 / amax))
    Fq = np.clip(F * scale, -240.0, 240.0).astype(ml_dtypes.float8_e4m3)
    return Fq, scale


def _in_maps(Fq):
    maps = []
    for units in ASSIGN:
        xc = np.concatenate([Fq[:, u * 128:(u + 1) * 128] for u in units],
                            axis=1)
        # device layout [half, partition, supertile, col]
        xd = np.ascontiguousarray(
            xc.reshape(KT, 2, 128, ROW).transpose(1, 2, 0, 3))
        maps.append({"x": xd})
    return maps


def _assemble(outs, svec, scale, c_out):
    inv = 1.0 / (scale * scale * c_out)
    quad = {}
    for c, units in enumerate(ASSIGN):
        o = outs[c].astype(np.float64) * inv
        col = 0
        for (s, mc, nw) in WINDOWS:
            su = units[s]
            block = o[:, col:col + nw]
            col += nw
            m0 = mc // 128
            for t in range(nw // 128):
                quad[(su, units[m0 + t])] = block[:, t * 128:(t + 1) * 128]
    loss = 0.0
    for i in range(10):
        s_i = np.concatenate([svec[2 * i], svec[2 * i + 1]])
        for j in range(i + 1, 10):
            s_j = np.concatenate([svec[2 * j], svec[2 * j + 1]])
            A = np.empty((256, 256))
            for a in range(2):
                for b in range(2):
                    u, v = 2 * i + a, 2 * j + b
                    q = quad[(u, v)] if (u, v) in quad else quad[(v, u)].T
                    A[a * 128:(a + 1) * 128, b * 128:(b + 1) * 128] = q
            C = A - np.outer(s_i, s_j) / float(N)
            loss += float((C * C).sum())
    loss /= float((N - 1) * (N - 1))
    return np.asarray([loss], np.float32)


def kernel(final_readout, weight, _trace=False):
    X = np.ascontiguousarray(np.asarray(final_readout, np.float32))
    w = np.asarray(weight, np.float32)
    Fq, scale = _quantize(X, w)
    # column sums of the quantized data (exact, fp64) for the centering
    # correction; must match the data the device saw.
    Fq64 = Fq.astype(np.float64)
    scol = Fq64.sum(axis=0) / scale
    svec = {u: scol[u * 128:(u + 1) * 128] for u in range(20)}
    # output rescale: |A_q| <= max col norm squared (Cauchy-Schwarz), so
    # A_q * c_out fits fp8e4 range with no clipping
    cn2 = float((Fq64 * Fq64).sum(axis=0).max())
    c_out = 2.0 ** np.floor(np.log2(224.0 / cn2)) if cn2 > 0 else 1.0
    nc = _get_nc(c_out)
    res = bass_utils.run_bass_kernel_spmd(
        nc, _in_maps(Fq), core_ids=list(range(8)), trace=_trace)
    _CACHE["last_results"] = res
    return _assemble([r["out"] for r in res.results], svec, scale, c_out)


# revision 30
# speedup vs baseline: 1.0378x; 1.0149x over previous
"""HSIC pairwise loss kernel for trn2 (8 NeuronCores), fp8 DoubleRow.

Math: reference builds K_c = (w^2 w^2T) * (E_c E_c^T), M_c = R K_c, and sums
tr(M_i M_j) over i<j. With F_c = w^2 * E_c (row scaling), R the centering
matrix (idempotent):
    tr(R K_i R K_j) = ||G_i^T G_j||_F^2,  G_c = F_c - colmean(F_c)
and with A_ij = F_i^T F_j, s_c = F_c^T 1:
    G_i^T G_j = A_ij - (1/n) s_i s_j^T
so loss = sum_{i<j} ||A_ij - s_i s_j^T / n||_F^2 / (n-1)^2.

Device work: the 45 A_ij blocks [256,256] (contraction over n=4096) at
half-chunk granularity: 180 unordered cross-parent pairs of the 20
128-col units.  Decomposition: at the PARENT (chunk) level, cover K10's
45 edges with 8 bipartite K2,3 graphs (one per core, exactly 3 edges
double-covered).  Core c loads its 2 left parents (4 units, slots 0-3)
and 3 right parents (6 units, slots 4-9) = 10 units, 5.2 MB fp8, and
computes all 24 left x right unit blocks as six 512-wide DoubleRow
windows into 6 PSUM banks (24 blocks/core vs the 26 of the previous
window-search assignment; 22.5 is the absolute lower bound, and 23 is
provably unreachable for any SPMD one-hole bipartite template).

Input DMA rides BOTH HWDGE rings (sync/SP h0 + scalar/ACT h1,
concurrent FIFO rings).  A DMA's completion semaphore fires ~2.5-4us
AFTER its transfer ends (write-receipt pipeline, deeper under load),
so first-consumable data is ~10.2-10.5us regardless of transfer
speed: the PE warm-up dummy ramp (vector-memset zero tile, 7 wide + 6
narrow DR matmuls, ~3.6us) bridges from body entry to exactly then,
and HAM (needing ~3.42us of SUSTAINED activity) unthrottles the PE to
2.4 GHz just before the real matmuls start.  Super-tiles 0 and 1 ride
one batched transfer per ring so both are consumable when the ramp
ends; k>=2 go as singles for fine wait granularity (more/smaller
transfers starve the 4 completion lanes per ring - measured).  The
PSUM drain runs on vector only (no ACTIVATE -> no 1.3us
ACT_TABLE_LOAD blocking the ACT ring at start); the last 4 supertiles
are peeled window-major (stop cadence 864ns > 678ns drain) so drains
and 3 of the 4 output DMAs cascade DURING the final matmuls.  Host
pre-scales w^2*X by a power of two into fp8e4 (loss rel-err ~2e-3,
tolerance 2e-2), takes column sums for the rank-1 centering
correction, and assembles/reduces in float64.

Measured: ~37.0-37.5us HW exec (prev session's kernel: 40.5us).
Fixed costs in the measured window: ~4.8us from window start to
first-consumable data, ~2.4us final-output-DMA completion wait, and
~7us NEFF per-engine semaphore-file zeroing epilogue + barriers
(lowering-emitted, present in any kernel).  The 20.9us matmul phase
is fp8-DoubleRow peak (157 TF/s) x 24/22.5 cover overhead.
"""

import numpy as np
import ml_dtypes
from contextlib import ExitStack

import concourse.bass as bass
import concourse.tile as tile
from concourse import bacc, mybir
from concourse import bass_utils

N = 4096
KT = 16                      # k super-tiles of 256 rows (DoubleRow)
UNITS = 10                   # units (128-col half-chunks) per core
ROW = UNITS * 128            # 1280 data cols
WARM_MMS = 7                 # wide dummy matmuls to pre-warm the PE (HAM)
WARM_MMS_NARROW = 6          # fine-grained tail of the warm-up ramp
PEEL = 4                     # trailing super-tiles run window-major so the
                             # PSUM drains cascade under the final matmuls

# Parent-level K2,3 cover of K10: core c loads left parents L (slots
# 0-3) and right parents R (slots 4-9); covers all L x R unit pairs.
COVER = [
    ((5, 7), (6, 4, 3)),
    ((7, 6), (2, 0, 8)),
    ((4, 5), (7, 1, 2)),
    ((6, 8), (4, 3, 5)),
    ((1, 9), (0, 6, 7)),
    ((8, 3), (4, 0, 1)),
    ((9, 0), (4, 2, 5)),
    ((9, 2), (3, 8, 1)),
]
ASSIGN = [
    [2 * p + h for p in L + R for h in (0, 1)] for (L, R) in COVER
]

# (stat_slot, moving_start_col, n_cols): slots 0-3 against slots 4-7,
# slots 8 and 9 against slots 0-3  ->  full K4,6 = 24 blocks.  The last
# 512 window is split 384+128 so the FINAL window's PSUM drain + output
# DMA (which sit after the last matmul on the critical path, ahead of a
# ~2.4us DMA-completion wait the epilogue barriers gate on) are tiny.
WINDOWS = [
    (0, 512, 512),
    (1, 512, 512),
    (2, 512, 512),
    (3, 512, 512),
    (8, 0, 512),
    (9, 0, 384),
    (3, 1152, 128),   # pair (3,9) with stationary 3: a DIFFERENT weight
                      # than w5's, so its LDWEIGHTS pipelines instead of
                      # serializing behind w5's matmul (same-address
                      # LDWEIGHTS cannot be pulled ahead)
]
OUT_COLS = sum(w[2] for w in WINDOWS)   # 3072

_CACHE = {}


def _build(c_out):
    f32 = mybir.dt.float32
    f8 = mybir.dt.float8e4
    DR = mybir.MatmulPerfMode.DoubleRow
    nc = bacc.Bacc("TRN2", target_bir_lowering=False, debug=False,
                   num_devices=8)
    # input layout [half, partition, supertile, col]: lets one DMA carry
    # one half of several supertiles as a single strided 3D transfer.
    x = nc.dram_tensor("x", [2, 128, KT, ROW], f8, kind="ExternalInput").ap()
    out = nc.dram_tensor("out", [128, OUT_COLS], f8,
                         kind="ExternalOutput").ap()

    with tile.TileContext(nc) as tc:
        with ExitStack() as ctx:
            zpool = ctx.enter_context(tc.tile_pool(name="z", bufs=1))
            xpool = ctx.enter_context(tc.tile_pool(name="xs", bufs=1))
            psum = ctx.enter_context(tc.tile_pool(name="ps", bufs=1,
                                                  space="PSUM"))
            opool = ctx.enter_context(tc.tile_pool(name="o", bufs=1))

            ps = []
            for i, (_, _, nw) in enumerate(WINDOWS):
                pst = psum.tile([128, nw], f32, tag=f"ps{i}", name=f"ps{i}")
                ps.append(pst)

            # PE warm-up: dummy DoubleRow matmuls start the HAM activity
            # window (~3.4us to unthrottle 1.2 -> 2.4 GHz).  The input
            # DMA completion semaphores fire only ~2.2-2.6us after the
            # transfer ends (~10.2-10.5us), so the ramp bridges from
            # body entry until then.  Products land in ps[0] and are
            # discarded (the real start=True matmul resets it).
            zt = zpool.tile([128, 2, 128], f8, tag="zt")
            nc.vector.memset(zt[:], 0.0)
            zr = zt[:, :, :].unsqueeze(2).broadcast_to([128, 2, 4, 128])
            for _ in range(WARM_MMS):
                nc.tensor.matmul(ps[0][:, 0:512], zt[:, :, :], zr,
                                 start=True, stop=True, perf_mode=DR)
            for _ in range(WARM_MMS_NARROW):
                nc.tensor.matmul(ps[0][:, 0:128], zt[:, :, :], zt[:, :, :],
                                 start=True, stop=True, perf_mode=DR)

            # Input: supertile halves on the two concurrent HWDGE rings
            # (sync h0 + scalar h1).  Super-tiles 0 and 1 ride ONE
            # transfer per ring (their completion semaphores fire
            # ~2.5-4us after the transfer ends, and a ring's 2nd
            # transfer completes ~1.9us after its 1st -- batching makes
            # BOTH supertiles consumable right as the warm-up ramp ends
            # instead of stalling the PE after supertile 0).  k>=2 go as
            # singles so the matmul wait granularity stays fine; more or
            # smaller transfers only deepen the completion queue and
            # starve the 4 in-flight lanes per ring (measured).
            fts = []
            for k in range(KT):
                ft = xpool.tile([128, 2, ROW], f8, name=f"ft{k}")
                nc.sync.dma_start(ft[:, 0, :], x[0, :, k, :])
                nc.scalar.dma_start(ft[:, 1, :], x[1, :, k, :])
                fts.append(ft[:, :, :])
            for k in range(KT - PEEL):
                ft = fts[k]
                for wi, (s, mc, nw) in enumerate(WINDOWS):
                    nc.tensor.matmul(
                        ps[wi][:, 0:nw],
                        ft[:, :, s * 128:(s + 1) * 128],
                        ft[:, :, mc:mc + nw],
                        start=(k == 0),
                        stop=False,
                        perf_mode=DR,
                    )
            for wi, (s, mc, nw) in enumerate(WINDOWS):
                for k in range(KT - PEEL, KT):
                    ft = fts[k]
                    nc.tensor.matmul(
                        ps[wi][:, 0:nw],
                        ft[:, :, s * 128:(s + 1) * 128],
                        ft[:, :, mc:mc + nw],
                        start=False,
                        stop=(k == KT - 1),
                        perf_mode=DR,
                    )

            # PSUM -> SBUF on vector only, scaled into fp8 range (c_out
            # is a power of two picked on the host so |A|*c_out <= 224
            # by Cauchy-Schwarz); three output DMAs so transfers overlap
            # the remaining copies.
            # Output cuts: the first three ride the sync ring while the
            # peel is still computing; the FINAL tiny cut (128 cols,
            # after the last matmul) goes on the otherwise-idle scalar
            # ring so its trigger is not queued behind a sync-ring
            # trigger still in flight (~600ns floor each).
            ot = opool.tile([128, OUT_COLS], f8)
            col = 0
            cuts = []
            for wi, (s, mc, nw) in enumerate(WINDOWS):
                nc.vector.tensor_scalar_mul(ot[:, col:col + nw],
                                            ps[wi][:, 0:nw], c_out)
                col += nw
                if wi in (1, 3, 5, 6):
                    cuts.append(col)
            lo = 0
            for ci, hi in enumerate(cuts):
                eng = nc.scalar if ci == len(cuts) - 1 else nc.sync
                eng.dma_start(out[:, lo:hi], ot[:, lo:hi])
                lo = hi
    nc.compile()
    return nc


def _get_nc(c_out):
    if _CACHE.get("c_out") != c_out:
        _CACHE["nc"] = _build(c_out)
        _CACHE["c_out"] = c_out
    return _CACHE["nc"]


def _quantize(X, w):
    """Host prep: F = w^2 * X, scaled by a power of two into fp8e4 range."""
    F = (w.astype(np.float64) ** 2) * X.astype(np.float64)
    amax = float(np.abs(F).max())
    if amax == 0.0 or not np.isfinite(amax):
        scale = 1.0
    else:
        scale = 2.0 ** np.floor(np.log2(192.0 / amax))
    Fq = np.clip(F * scale, -240.0, 240.0).astype(ml_dtypes.float8_e4m3)
    return Fq, scale


def _in_maps(Fq):
    maps = []
    for units in ASSIGN:
        xc = np.concatenate([Fq[:, u * 128:(u + 1) * 128] for u in units],
                            axis=1)
        # device layout [half, partition, supertile, col]
        xd = np.ascontiguousarray(
            xc.reshape(KT, 2, 128, ROW).transpose(1, 2, 0, 3))
        maps.append({"x": xd})
    return maps


def _assemble(outs, svec, scale, c_out):
    inv = 1.0 / (scale * scale * c_out)
    quad = {}
    for c, units in enumerate(ASSIGN):
        o = outs[c].astype(np.float64) * inv
        col = 0
        for (s, mc, nw) in WINDOWS:
            su = units[s]
            block = o[:, col:col + nw]
            col += nw
            m0 = mc // 128
            for t in range(nw // 128):
                quad[(su, units[m0 + t])] = block[:, t * 128:(t + 1) * 128]
    loss = 0.0
    for i in range(10):
        s_i = np.concatenate([svec[2 * i], svec[2 * i + 1]])
        for j in range(i + 1, 10):
            s_j = np.concatenate([svec[2 * j], svec[2 * j + 1]])
            A = np.empty((256, 256))
            for a in range(2):
                for b in range(2):
                    u, v = 2 * i + a, 2 * j + b
                    q = quad[(u, v)] if (u, v) in quad else quad[(v, u)].T
                    A[a * 128:(a + 1) * 128, b * 128:(b + 1) * 128] = q
            C = A - np.outer(s_i, s_j) / float(N)
            loss += float((C * C).sum())
    loss /= float((N - 1) * (N - 1))
    return np.asarray([loss], np.float32)


def kernel(final_readout, weight, _trace=False):
    X = np.ascontiguousarray(np.asarray(final_readout, np.float32))
    w = np.asarray(weight, np.float32)
    Fq, scale = _quantize(X, w)
    # column sums of the quantized data (exact, fp64) for the centering
    # correction; must match the data the device saw.
    Fq64 = Fq.astype(np.float64)
    scol = Fq64.sum(axis=0) / scale
    svec = {u: scol[u * 128:(u + 1) * 128] for u in range(20)}
    # output rescale: |A_q| <= max col norm squared (Cauchy-Schwarz), so
    # A_q * c_out fits fp8e4 range with no clipping
    cn2 = float((Fq64 * Fq64).sum(axis=0).max())
    c_out = 2.0 ** np.floor(np.log2(224.0 / cn2)) if cn2 > 0 else 1.0
    nc = _get_nc(c_out)
    res = bass_utils.run_bass_kernel_spmd(
        nc, _in_maps(Fq), core_ids=list(range(8)), trace=_trace)
    _CACHE["last_results"] = res
    return _assemble([r["out"] for r in res.results], svec, scale, c_out)
